# revision 2
# baseline (speedup 1.0000x reference)
"""BinsChamferLoss Trainium2 kernel (v2: PE-generated diffs).

Per core, 4 patches of (12544 points x 256 centers):
  PE   : 98 K=2 float32r matmuls per patch produce diff[q,i] = c_i - p~_q
         (operands pre-rounded to fp22; p~ = p + 200*(p<=0) so invalid
         points never win a min; fp32 PSUM accumulate)
  ACT  : 17 wide Square instructions per patch cast PSUM fp32 -> SBUF fp16
  DVE  : cham_y: per 12/14-tile supertile, four 2x-rate half-folds
         (256->16) + one strided tensor_reduce -> per-point min over centers
         cham_x: fp16 min-accumulate over superblocks + fold + PE transpose
         + free-axis min -> per-center min over valid points
  prep : mask/substitute on (128,98), PE transpose, flatten DMAs into the
         (2, 12544) float32r lhsT rows (row1 = 1s)
  NOTE : tensor_tensor_reduce and gpsimd partition_all_reduce hang real
         HW through this toolchain; both default OFF (plain mult+reduce
         and a ones-column PE matmul are used instead).
The host combines per-patch A/B/C exactly as v1: mean over patches of
(counts>0) ? A/256 + B/max(counts,1) : 0.
"""

import os
from contextlib import ExitStack

import numpy as np

KP = 112
Q = KP * KP            # 12544
NPART = 128
NT = Q // NPART        # 98 point tiles
PC = 256               # centers
BIGP = 200.0
SBT = 6                # tiles per PSUM superblock (3 banks; x2 bufs = 6)
# supertiles: 7 groups of 12 tiles (2 superblocks) + 1 of 14 (2 SBs + partial)
GROUPS = [12] * 7 + [14]

N_CORES = 8
PATCHES_PER_CORE = 4

# engine-split tunables. NOTE: generic tensor_tensor is NOT a legal opcode on
# the Pool/gpsimd engine (walrus codegen rejects it), so min-offload to
# gpsimd is impossible; both stay 0. gpsimd still runs SWDGE DMAs and the
# partition_all_reduce custom op.
POOL_CHX_SBS = 0
POOL_U1_GRPS = 0


def _build_module(loop_n=None, pool_chx=POOL_CHX_SBS, pool_u1=POOL_U1_GRPS,
                     reps=1, gp_dma=True, gp_par=False, use_ttr=False):
    import concourse.bass as bass
    import concourse.tile as tile
    from concourse import bacc, mybir
    from concourse import bass_isa
    from concourse.masks import make_identity

    f32 = mybir.dt.float32
    f32r = mybir.dt.float32r
    f16 = mybir.dt.float16
    Alu = mybir.AluOpType
    Act = mybir.ActivationFunctionType
    X = mybir.AxisListType.X

    nc = bacc.Bacc("TRN2", target_bir_lowering=False, debug=False,
                   num_devices=N_CORES)

    bins4 = nc.dram_tensor("bins4", (257, PATCHES_PER_CORE), f32,
                           kind="ExternalInput").ap()
    pts = nc.dram_tensor("pts", (PATCHES_PER_CORE, NPART, NT), f32,
                         kind="ExternalInput").ap()
    outv = nc.dram_tensor("outv", (1, 3 * PATCHES_PER_CORE), f32,
                          kind="ExternalOutput").ap()

    with tile.TileContext(nc) as tc, ExitStack() as ctx:
        const_pool = ctx.enter_context(tc.tile_pool(name="const", bufs=1))
        bins_pool = ctx.enter_context(tc.tile_pool(name="bins", bufs=1))
        prep_pool = ctx.enter_context(tc.tile_pool(name="prep", bufs=2))
        d8_pool = ctx.enter_context(tc.tile_pool(name="d8", bufs=4))
        u_pool = ctx.enter_context(tc.tile_pool(name="u", bufs=3))
        acc_pool = ctx.enter_context(tc.tile_pool(name="acc", bufs=2))
        mrg_pool = ctx.enter_context(tc.tile_pool(name="mrg", bufs=2))
        res_pool = ctx.enter_context(tc.tile_pool(name="res", bufs=1))

        ps_mm = ctx.enter_context(tc.tile_pool(name="ps_mm", bufs=2,
                                               space="PSUM"))
        ps_tr = ctx.enter_context(tc.tile_pool(name="ps_tr", bufs=1,
                                               space="PSUM"))
        ps_tail = ctx.enter_context(tc.tile_pool(name="ps_tail", bufs=1,
                                                 space="PSUM"))

        # ---- constants ------------------------------------------------
        ident = const_pool.tile([128, 128], f32)
        make_identity(nc, ident[:])
        ident16 = const_pool.tile([128, 128], f16)
        make_identity(nc, ident16[:])
        ones_col = const_pool.tile([128, 1], f32)
        nc.vector.memset(ones_col[:], 1.0)

        # rhs for the diff matmuls: row0 = -1s, row1 = centers (4 patches).
        # float32r-typed: fp32r matmul operands must be produced pre-rounded
        # to fp22 or the BIR verifier rejects / HW misbehaves.
        u32 = mybir.dt.uint32
        crow4 = const_pool.tile([2, PATCHES_PER_CORE * PC], f32r)
        nc.vector.memset(crow4[:].bitcast(u32), 0xBF800000)  # -1.0f

        # flat point rows, manual A/B double buffer: row0 = p~ flat,
        # row1 = +1s (lhsT contraction row)
        # memset both partitions to 1.0 (single-partition compute access at
        # base partition 1 fails BIR verification); row 0 is overwritten by
        # the per-patch flatten DMAs, row 1 stays 1.0 = the contraction row.
        pts2a = const_pool.tile([2, Q], f32r, tag="pts2a")
        pts2b = const_pool.tile([2, Q], f32r, tag="pts2b")
        nc.vector.memset(pts2a[:].bitcast(u32), 0x3F800000)  # 1.0f
        nc.vector.memset(pts2b[:].bitcast(u32), 0x3F800000)

        # ---- centers (same as v1) -------------------------------------
        b_lo0 = bins_pool.tile([128, PATCHES_PER_CORE], f32, tag="b0")
        b_lo1 = bins_pool.tile([128, PATCHES_PER_CORE], f32, tag="b1")
        b_hi0 = bins_pool.tile([128, PATCHES_PER_CORE], f32, tag="b2")
        b_hi1 = bins_pool.tile([128, PATCHES_PER_CORE], f32, tag="b3")
        nc.sync.dma_start(b_lo0[:], bins4[0:128, :])
        nc.sync.dma_start(b_lo1[:], bins4[1:129, :])
        nc.sync.dma_start(b_hi0[:], bins4[128:256, :])
        nc.sync.dma_start(b_hi1[:], bins4[129:257, :])
        ch0 = bins_pool.tile([128, PATCHES_PER_CORE], f32, tag="ch0")
        ch1 = bins_pool.tile([128, PATCHES_PER_CORE], f32, tag="ch1")
        nc.vector.tensor_add(ch0[:], b_lo0[:], b_lo1[:])
        nc.vector.tensor_scalar_mul(ch0[:], ch0[:], 0.5)
        nc.vector.tensor_add(ch1[:], b_hi0[:], b_hi1[:])
        nc.vector.tensor_scalar_mul(ch1[:], ch1[:], 0.5)
        pt0 = ps_tr.tile([PATCHES_PER_CORE, 128], f32, tag="trA")
        pt1 = ps_tr.tile([PATCHES_PER_CORE, 128], f32, tag="trA")
        nc.tensor.transpose(pt0[:], ch0[:], ident[:])
        nc.tensor.transpose(pt1[:], ch1[:], ident[:])
        cT = bins_pool.tile([PATCHES_PER_CORE, PC], f32r, tag="cT")
        nc.vector.tensor_copy(cT[:, 0:128], pt0[:])
        nc.vector.tensor_copy(cT[:, 128:256], pt1[:])
        # flatten (4,256) -> (1,1024) into crow4 row 1
        nc.sync.dma_start(crow4[1:2, :], cT[:])

        results = res_pool.tile([1, 3 * PATCHES_PER_CORE], f32)

        loop_ctx = (tc.For_i(0, loop_n, 1,
                             hint_engines=(mybir.EngineType.Activation,
                                           mybir.EngineType.DVE))
                    if loop_n is not None else None)
        if loop_ctx is not None:
            ctx.enter_context(loop_ctx)

        for k in [k for _ in range(reps) for k in range(PATCHES_PER_CORE)]:
            pts2 = pts2a if k % 2 == 0 else pts2b

            # ---- prep: mask + p~ + flat row --------------------------
            p0 = prep_pool.tile([NPART, NT], f32, tag="p0")
            nc.sync.dma_start(p0[:], pts[k])
            msk = prep_pool.tile([NPART, NT], f32, tag="msk")
            nc.vector.tensor_scalar(msk[:], p0[:], 0.0, None, op0=Alu.is_gt)
            inv = prep_pool.tile([NPART, NT], f32, tag="inv")
            nc.vector.tensor_scalar(inv[:], p0[:], 0.0, None, op0=Alu.is_le)
            ptld = prep_pool.tile([NPART, NT], f32, tag="ptld")
            nc.vector.scalar_tensor_tensor(ptld[:], inv[:], BIGP, p0[:],
                                           op0=Alu.mult, op1=Alu.add)
            ccol = prep_pool.tile([NPART, 1], f32, tag="ccol")
            nc.vector.tensor_reduce(ccol[:], msk[:], axis=X, op=Alu.add)

            ptT_ps = ps_tr.tile([NT, 128], f32, tag="trA")
            nc.tensor.transpose(ptT_ps[:], ptld[:], ident[:])
            ptldT = prep_pool.tile([NT, 128], f32r, tag="ptldT")
            nc.vector.tensor_copy(ptldT[:], ptT_ps[:])
            # flatten on scalar(+gpsimd) DGE queues: SP stays free so the
            # next patch's p0 load DMA is never stuck behind these in a FIFO
            dma_engs = [nc.scalar, nc.gpsimd] if gp_dma else [nc.scalar]
            for c in range(7):
                dma_engs[c % len(dma_engs)].dma_start(
                    pts2[0:1, c * 14 * 128:(c + 1) * 14 * 128],
                    ptldT[c * 14:(c + 1) * 14, :])

            crow = crow4[:, k * PC:(k + 1) * PC]

            minx = prep_pool.tile([NPART, NT], f32, tag="minx")
            accD = acc_pool.tile([NPART, SBT * PC], f16, tag="accD")
            accP = None
            if pool_chx:
                accP = acc_pool.tile([NPART, SBT * PC], f16, tag="accP")
            nD = nP = 0
            n_sbs = (NT + SBT - 1) // SBT  # 17 (16 full + 1 of 2 tiles)

            # ---- main loop over supertiles ----------------------------
            col0 = 0
            for g, gtiles in enumerate(GROUPS):
                d8s = d8_pool.tile([NPART, gtiles * PC], f16, tag="d8s")
                nsb_g = (gtiles + SBT - 1) // SBT
                for sb in range(nsb_g):
                    sbt = min(SBT, gtiles - sb * SBT)
                    ps = ps_mm.tile([NPART, SBT * PC], f32, tag="ps")
                    for t in range(sbt):
                        col = col0 + sb * SBT + t
                        nc.tensor.matmul(
                            ps[:, t * PC:(t + 1) * PC],
                            pts2[:, col * 128:(col + 1) * 128],
                            crow,
                            start=True, stop=True)
                    src = d8s[:, sb * SBT * PC:(sb * SBT + sbt) * PC]
                    nc.scalar.activation(src, ps[:, 0:sbt * PC], Act.Square)
                    # cham_x: accumulate nearest-point mins per center.
                    # gpsimd (slow chain) gets the EARLY superblocks so its
                    # serial chain drains by mid-patch; DVE absorbs the rest.
                    # Init via DVE copy (4x) -- memset is 1x-rate.
                    sb_idx = nD + nP
                    if sb_idx < pool_chx:
                        eng, acc, first = nc.gpsimd, accP, nP == 0
                        nP += 1
                    else:
                        eng, acc, first = nc.vector, accD, nD == 0
                        nD += 1
                    if first:
                        assert sbt == SBT
                        nc.vector.tensor_copy(acc[:], src)
                    else:
                        eng.tensor_tensor(acc[:, 0:sbt * PC],
                                          acc[:, 0:sbt * PC], src, op=Alu.min)

                # cham_y: fold 256 -> 16 at 2x rate, then one 1x reduce
                dv = d8s[:].rearrange("p (j c) -> p j c", c=PC)
                u1 = u_pool.tile([NPART, 14 * 128], f16, tag="u1")
                w1 = u1[:, 0:gtiles * 128].rearrange("p (j c) -> p j c", c=128)
                nc.vector.tensor_tensor(w1, dv[:, :, 0:128], dv[:, :, 128:256],
                                        op=Alu.min)
                u2 = u_pool.tile([NPART, 14 * 64], f16, tag="u2")
                w2 = u2[:, 0:gtiles * 64].rearrange("p (j c) -> p j c", c=64)
                nc.vector.tensor_tensor(w2, w1[:, :, 0:64], w1[:, :, 64:128],
                                        op=Alu.min)
                u3 = u_pool.tile([NPART, 14 * 32], f16, tag="u3")
                w3 = u3[:, 0:gtiles * 32].rearrange("p (j c) -> p j c", c=32)
                nc.vector.tensor_tensor(w3, w2[:, :, 0:32], w2[:, :, 32:64],
                                        op=Alu.min)
                u4 = u_pool.tile([NPART, 14 * 16], f16, tag="u4")
                w4 = u4[:, 0:gtiles * 16].rearrange("p (j c) -> p j c", c=16)
                nc.vector.tensor_tensor(w4, w3[:, :, 0:16], w3[:, :, 16:32],
                                        op=Alu.min)
                nc.vector.tensor_reduce(minx[:, col0:col0 + gtiles], w4,
                                        axis=X, op=Alu.min)
                col0 += gtiles

            # ---- cham_x tail: fold 7 blocks, min over points ----------
            if pool_chx:
                nc.vector.tensor_tensor(accD[:], accD[:], accP[:], op=Alu.min)
            m4 = mrg_pool.tile([NPART, 3 * PC], f16, tag="m4")
            nc.vector.tensor_tensor(m4[:], accD[:, 0:3 * PC],
                                    accD[:, 3 * PC:6 * PC], op=Alu.min)
            m2 = mrg_pool.tile([NPART, PC], f16, tag="m2")
            nc.vector.tensor_tensor(m2[:], m4[:, 0:PC], m4[:, PC:2 * PC],
                                    op=Alu.min)
            accf = mrg_pool.tile([NPART, PC], f16, tag="accf")
            nc.vector.tensor_tensor(accf[:], m2[:], m4[:, 2 * PC:3 * PC],
                                    op=Alu.min)

            trA = ps_tail.tile([128, 128], f16, tag="trT")
            nc.tensor.transpose(trA[:], accf[:, 0:128], ident16[:])
            miny0 = mrg_pool.tile([128, 1], f32, tag="miny0")
            nc.vector.tensor_reduce(miny0[:], trA[:], axis=X, op=Alu.min)
            trB = ps_tail.tile([128, 128], f16, tag="trT")
            nc.tensor.transpose(trB[:], accf[:, 128:256], ident16[:])
            miny1 = mrg_pool.tile([128, 1], f32, tag="miny1")
            nc.vector.tensor_reduce(miny1[:], trB[:], axis=X, op=Alu.min)
            abc = prep_pool.tile([NPART, 3], f32, tag="abc")
            nc.vector.tensor_add(abc[:, 0:1], miny0[:], miny1[:])

            # ---- cham_y tail: B = sum m*minx --------------------------
            w = prep_pool.tile([NPART, NT], f32, tag="w")
            if use_ttr:
                nc.vector.tensor_tensor_reduce(
                    out=w[:], in0=minx[:], in1=msk[:], scale=1.0, scalar=0.0,
                    op0=Alu.mult, op1=Alu.add, accum_out=abc[:, 1:2])
            else:
                nc.vector.tensor_tensor(w[:], minx[:], msk[:], op=Alu.mult)
                nc.vector.tensor_reduce(abc[:, 1:2], w[:], axis=X, op=Alu.add)
            nc.vector.tensor_copy(abc[:, 2:3], ccol[:])
            if gp_par:
                # A/B/C partition sums in one gpsimd all-reduce (no PSUM/PE)
                abr = prep_pool.tile([NPART, 3], f32, tag="abr")
                nc.gpsimd.partition_all_reduce(abr[:], abc[:], channels=NPART,
                                               reduce_op=bass_isa.ReduceOp.add)
                nc.vector.tensor_copy(results[:, 3 * k:3 * k + 3], abr[0:1, :])
            else:
                # fallback: ones-column matmul sums partitions -> (1, 3)
                ps3 = ps_tail.tile([1, 3], f32, tag="trT")
                nc.tensor.matmul(ps3[:], ones_col[:], abc[:],
                                 start=True, stop=True)
                nc.vector.tensor_copy(results[:, 3 * k:3 * k + 3], ps3[:])

        nc.sync.dma_start(outv[:], results[:])

    nc.finalize()
    return nc


_NC_CACHE = {}


def _get_module(reps=1):
    key = ("nc", reps)
    if key not in _NC_CACHE:
        _NC_CACHE[key] = _build_module(reps=reps)
    return _NC_CACHE[key]


def _make_exec(nc):
    """Build a reusable jitted executor for the 8-core SPMD module.

    Mirrors concourse.bass2jax.run_bass_via_pjrt's multi-core branch but
    returns a callable so repeated executions reuse the compiled NEFF.
    """
    key = ("exec", id(nc))
    if key in _NC_CACHE:
        return _NC_CACHE[key]
    import jax
    import numpy as _np
    from jax.sharding import Mesh, PartitionSpec
    from jax.experimental.shard_map import shard_map
    from concourse import mybir
    from concourse import bass2jax as b2j

    b2j.install_neuronx_cc_hook()
    partition_name = (nc.partition_id_tensor.name
                      if nc.partition_id_tensor else None)
    in_names, out_names, out_avals, zero_outs = [], [], [], []
    for alloc in nc.m.functions[0].allocations:
        if not isinstance(alloc, mybir.MemoryLocationSet):
            continue
        name = alloc.memorylocations[0].name
        if alloc.kind == "ExternalInput":
            if name != partition_name:
                in_names.append(name)
        elif alloc.kind == "ExternalOutput":
            shape = tuple(alloc.tensor_shape)
            dtype = mybir.dt.np(alloc.dtype)
            out_names.append(name)
            out_avals.append(jax.core.ShapedArray(shape, dtype))
            zero_outs.append(_np.zeros(shape, dtype))
    n_params = len(in_names)
    n_outs = len(out_avals)
    all_in_names = tuple(in_names + out_names +
                         ([partition_name] if partition_name else []))
    donate = tuple(range(n_params, n_params + n_outs))

    def _body(*args):
        operands = list(args)
        if partition_name is not None:
            operands.append(b2j.partition_id_tensor())
        outs = b2j._bass_exec_p.bind(
            *operands,
            out_avals=tuple(out_avals),
            in_names=all_in_names,
            out_names=tuple(out_names),
            lowering_input_output_aliases=(),
            sim_require_finite=True,
            sim_require_nnan=True,
            nc=nc,
        )
        return tuple(outs)

    devices = jax.devices()[:N_CORES]
    mesh = Mesh(_np.asarray(devices), ("core",))
    in_specs = (PartitionSpec("core"),) * (n_params + n_outs)
    out_specs = (PartitionSpec("core"),) * n_outs
    sharded = jax.jit(
        shard_map(_body, mesh=mesh, in_specs=in_specs, out_specs=out_specs,
                  check_rep=False),
        donate_argnums=donate, keep_unused=True)

    def execute(in_maps, block=True):
        per_core = [[_np.asarray(m[name]) for name in in_names]
                    for m in in_maps]
        concat_in = [
            _np.concatenate([per_core[c][i] for c in range(N_CORES)], axis=0)
            for i in range(n_params)
        ]
        concat_zeros = [
            _np.zeros((N_CORES * z.shape[0], *z.shape[1:]), z.dtype)
            for z in zero_outs
        ]
        out_arrs = sharded(*concat_in, *concat_zeros)
        if block:
            jax.block_until_ready(out_arrs)
        return [
            {name: _np.asarray(out_arrs[i]).reshape(
                N_CORES, *out_avals[i].shape)[c]
             for i, name in enumerate(out_names)}
            for c in range(N_CORES)
        ]

    _NC_CACHE[key] = execute
    return execute


def _shard_inputs(bins, target_depth_maps):
    bins = np.ascontiguousarray(np.asarray(bins, dtype=np.float32)).reshape(2, 257, 16)
    tgt = np.ascontiguousarray(
        np.asarray(target_depth_maps, dtype=np.float32)).reshape(2, 448, 448)
    in_maps = []
    for c in range(N_CORES):
        ids = [4 * c + j for j in range(PATCHES_PER_CORE)]
        n = ids[0] // 16
        ls = [i % 16 for i in ids]
        bins4 = np.ascontiguousarray(bins[n][:, ls])           # (257, 4)
        blocks = []
        for l in ls:
            hb, wb = l // 4, l % 4
            blk = tgt[n, hb * 112:(hb + 1) * 112, wb * 112:(wb + 1) * 112]
            blocks.append(np.ascontiguousarray(blk).reshape(NPART, NT))
        pts = np.stack(blocks)                                  # (4, 128, 98)
        in_maps.append({"bins4": bins4, "pts": np.ascontiguousarray(pts)})
    return in_maps


def _combine(results):
    per_patch = []
    for c in range(N_CORES):
        vals = np.asarray(results[c]["outv"], dtype=np.float64).reshape(
            PATCHES_PER_CORE, 3)
        for k in range(PATCHES_PER_CORE):
            A, B, C = vals[k]
            if C > 0:
                per_patch.append(A / PC + B / max(C, 1.0))
            else:
                per_patch.append(0.0)
    return np.float32(np.mean(np.asarray(per_patch, dtype=np.float64)))


def run(inputs, reps=1):
    nc = _get_module(reps)
    execute = _make_exec(nc)
    in_maps = _shard_inputs(**inputs)
    results = execute(in_maps)
    val = _combine(results)
    return val, execute, in_maps


def kernel(**inputs) -> np.ndarray:
    val, _, _ = run(inputs)
    return np.array(val, dtype=np.float32)

